# revision 39
# baseline (speedup 1.0000x reference)
"""GCN (2-layer, hidden=64, rank-1 weights) on 8 Trainium2 NeuronCores.

Math: both GCNConv layers have rank-1 weight matrices (1->64, 64->1), so each
layer collapses to a scalar SpMV with the symmetric-normalized adjacency
A_hat = D^-1/2 (A+I) D^-1/2:

    s   = A_hat @ x                    (scalar per node)
    z   = f(s)   where f(t) = sum_k W2[k] * relu(W1[k]*t + b1[k])
    out = A_hat @ z + b2

Sharding: nodes are range-sharded by destination across the 8 cores; all
in-edges of a node live on its owner core.  Within a core, nodes are sorted
by in-degree (descending); sorted-rank j maps to SBUF slot
(partition, column) = (j % 128, j // 128).  Round r (the r-th in-edge of
every node; the self-loop of node j occupies round deg(j), its first free
slot) is then a fully dense [128, w_r] block with w_r = ceil(n_r/128) --
the ELL is packed modulo-128 with no partition-height padding, so the DMA
moves only the live ~43% of the rectangular ELL and the fold matmuls only
stream live columns.

Round blocks are concatenated and split into column chunks
([round 0 | fat middle x2 | narrow tail]); each chunk is ONE fp16 DMA of
routed pre-normalized source values, issued from the two HWDGE rings
(sync/scalar engines) in parallel and consumed in landing order: the
round-0 chunk (whose full-width matmul resets the PSUM tile) and the tail
chunk (many narrow, issue-bound matmuls) land first and are folded while
the fat middle chunks are still in flight.

Per launch the segment-sum runs on the TENSOR engine: R accumulating
identity-matmuls (fp16 moving data, f32 PSUM accumulation into F[:, :w_r]),
pipelined behind the chunk DMAs; the identity is generated on the idle
GpSimd engine.  The per-node epilogue (dinv_dst scaling + the rank-1 MLP
folded to a 2-segment piecewise-linear map when b1 == 0) is 4 small
[128, 98] Vector ops.

Host preprocessing (the "halo exchange of gathered source features"):
degree tables (bincount, 1/sqrt(deg+1)) and the per-node normalized
message table y = dinv * x are computed per node, then routed per edge
into the packed ELL slots; between launches the device-computed per-node
w = dinv * f(s) values are routed the same way for layer 2.  The per-edge
aggregation (two 1.3M-element segment-sums) and all per-node nonlinear
math run on the NeuronCores.
"""

import os
import numpy as np

from concourse import bass, mybir
from concourse.bass_utils import run_bass_kernel_spmd

dt = mybir.dt

NCORES = 8
N = 100000
P = 128            # SBUF partitions
CPN = 98           # node columns per partition
NPC = P * CPN      # 12544 nodes per core
SENT = NCORES * NPC  # sentinel table slot (value 0)
NCH = 4            # DMA chunks per ell tensor

LAST_RESULTS = None  # list of BassKernelResults from the most recent run


def _preprocess(x, edge_index):
    """Host routing/layout: shard by destination, degree-sort nodes, build
    the modulo-128 packed ELL index layout and the chunk split."""
    x = np.asarray(x, dtype=np.float32).reshape(-1)
    ei = np.asarray(edge_index)
    src_g = ei[0].astype(np.int64)
    dst_g = ei[1].astype(np.int64)

    cnt_g = np.bincount(dst_g, minlength=N).astype(np.int64)  # in-degree
    dinv_g = 1.0 / np.sqrt(cnt_g.astype(np.float64) + 1.0)    # incl self-loop

    order_c, rank_c, deg_sorted_c = [], [], []
    pp = np.empty(N, dtype=np.int64)  # global node -> permuted table position
    for c in range(NCORES):
        lo, hi = c * NPC, min((c + 1) * NPC, N)
        nreal = hi - lo
        deg_local = np.full(NPC, -1, dtype=np.int64)  # pad slots: no self-loop
        deg_local[:nreal] = cnt_g[lo:hi]
        order = np.argsort(-deg_local, kind="stable")
        rank = np.empty(NPC, dtype=np.int64)
        rank[order] = np.arange(NPC)
        order_c.append(order)
        rank_c.append(rank)
        deg_sorted_c.append(deg_local[order])
        pp[lo:hi] = c * NPC + rank[:nreal]

    K = int(max(int(d[0]) for d in deg_sorted_c))  # global max in-degree
    R = K + 1  # +1 round absorbs the self-loops

    # per-round packed widths (max across cores, shared program shape)
    w_r = np.zeros(R, dtype=np.int64)
    for c in range(NCORES):
        ds = deg_sorted_c[c]
        for r in range(R):
            n_r = int(np.count_nonzero(ds >= r))
            w_r[r] = max(w_r[r], (n_r + P - 1) // P)
    w_r = np.maximum(w_r, 1)
    w_r[0] = CPN  # full width: the first matmul resets the whole PSUM tile
    offs_r = np.concatenate([[0], np.cumsum(w_r)])  # round col offsets
    W = int(offs_r[-1])

    # chunk split: [round 0 | fat middle x2 | skinny tail].  Round 0 alone
    # resets the full PSUM tile and lands first; the skinny tail's many
    # narrow (issue-bound) matmuls run while the fat chunks are in flight.
    sk = R
    for r in range(1, R):
        if w_r[r] <= 16:
            sk = r
            break
    if R >= 4 and sk - 1 >= 2:
        fat_cols = int(offs_r[sk] - offs_r[1])
        mid = sk
        acc = 0
        for r in range(1, sk):
            acc += int(w_r[r])
            if acc >= fat_cols / 2:
                mid = r + 1
                break
        mid = min(mid, sk - 1) if sk - 1 > 1 else sk
        chunks = [(0, 1), (1, mid), (mid, sk)]
        if sk < R:
            chunks.append((sk, R))
    else:
        step = max(1, R // NCH)
        chunks = [(a, min(a + step, R)) for a in range(0, R, step)]
        chunks[-1] = (chunks[-1][0], R)

    owner = dst_g // NPC
    idx_c, dinv_own_c = [], []
    for c in range(NCORES):
        lo = c * NPC
        m = owner == c
        s_e = pp[src_g[m]]
        d_e = dst_g[m] - lo
        rj = rank_c[c][d_e]
        o = np.argsort(rj, kind="stable")
        rj_s = rj[o]
        s_s = s_e[o]
        occ = np.arange(len(rj_s)) - np.searchsorted(rj_s, rj_s)
        idx_mat = np.full((NPC, R), SENT, dtype=np.int64)
        idx_mat[rj_s, occ] = s_s
        # self-loop of sorted-node j at round deg(j) (first free slot)
        nreal = min(NPC, N - lo)
        jreal = rank_c[c][:nreal]          # sorted positions of real nodes
        idx_mat[jreal, deg_sorted_c[c][jreal]] = lo + jreal
        # modulo-128 packed layout: rank j -> (j % P, offs_r[r] + j // P)
        idx_lay = np.full((P, W), SENT, dtype=np.int64)
        for r in range(R):
            w = int(w_r[r])
            idx_lay[:, offs_r[r]:offs_r[r] + w] = \
                idx_mat[:w * P, r].reshape(w, P).T
        idx_c.append(idx_lay)

        dv = np.zeros(NPC, dtype=np.float32)
        dv[:nreal] = dinv_g[lo:lo + nreal]
        dv_sorted = dv[order_c[c]]
        dinv_own_c.append(np.ascontiguousarray(
            dv_sorted.reshape(CPN, P).T.astype(np.float32)))

    return idx_c, dinv_own_c, rank_c, dinv_g, R, w_r, offs_r, chunks


def _build(R, w_r, offs_r, chunks, *, layer1, A=0.0, B=0.0, b2=0.0,
           terms=None, out_fp16=False):
    """One GCN layer over the modulo-128 packed ELL.

    chunks: list of (r0, r1) round ranges; chunk i's DMA carries the fp16
    columns offs_r[r0]:offs_r[r1] of routed, pre-normalized source values
    (dinv[src]*x[src] for layer 1, w[src] for layer 2).  The fold is pure
    accumulating matmuls either way; only the epilogue differs.
    """
    nc = bass.Bass(num_devices=NCORES, enable_partition_id=False)
    ncl = len(chunks)
    cw = [int(offs_r[r1] - offs_r[r0]) for (r0, r1) in chunks]
    coff = [int(offs_r[r0]) for (r0, r1) in chunks]
    total = int(offs_r[-1])

    # ring assignment and processing order (landing order); chunk 0 (which
    # contains round 0 and resets the PSUM) is always processed first
    if ncl == 4:
        sync_chunks = [0, 1]       # reset chunk + fat half a
        scal_chunks = [3, 2]       # skinny tail first, then fat half b
        order = [0, 3, 1, 2]
    else:
        sync_chunks = [i for i in range(ncl) if i % 2 == 0]
        scal_chunks = [i for i in range(ncl) if i % 2 == 1]
        order = [i for pair in zip(sync_chunks, scal_chunks) for i in pair]
        order += [i for i in range(ncl) if i not in order]
        order.remove(0)
        order = [0] + order
    xd_in = [nc.declare_dram_parameter(
        f"xd{i}", [P, cw[i]], dt.float16, isOutput=False)
        for i in range(ncl)]
    dn_in = nc.declare_dram_parameter("dn", [P, CPN], dt.float32, isOutput=False)
    out_dt = dt.float16 if out_fp16 else dt.float32
    out_ext = nc.declare_dram_parameter("out", [P, CPN], out_dt, isOutput=True)

    with (
        nc.sbuf_tensor("XD", [P, total], dt.float16) as XD,
        nc.sbuf_tensor("ID", [P, P], dt.float16) as ID,
        nc.sbuf_tensor("DN", [P, CPN], dt.float32) as DN,
        nc.sbuf_tensor("S", [P, CPN], dt.float32) as S,
        nc.sbuf_tensor("T", [P, CPN], dt.float32) as T,
        nc.sbuf_tensor("U", [P, CPN], dt.float32) as U,
        nc.sbuf_tensor("W", [P, CPN], out_dt) as W,
        nc.psum_tensor("F", [P, CPN], dt.float32) as F,
        nc.semaphore("si") as si,      # identity built (gpsimd)
        nc.semaphore("sn") as sn,      # dn loaded
        nc.semaphore("sv") as sv,      # DVE progress
        nc.semaphore("st") as st,      # PE fold done
        nc.semaphore("so") as so,      # out store
        nc.Block(no_gpsimd_drain=True) as block,
    ):
        sch = [nc.semaphore(f"sc{i}").__enter__() for i in range(ncl)]

        sv_n = [0]

        def v_inc(inst):
            inst.then_inc(sv, 1)
            sv_n[0] += 1
            return sv_n[0]

        # GpSimd: build the identity while the chunk DMAs land
        @block.gpsimd
        def _(gpsimd):
            gpsimd.memset(ID[:, :], 0.0)
            gpsimd.affine_select(
                out=ID[:, :], in_=ID[:, :],
                compare_op=mybir.AluOpType.not_equal,
                fill=1.0, base=0, pattern=[[-1, P]],
                channel_multiplier=1).then_inc(si, 1)

        @block.sync
        def _(sync):
            for i in sync_chunks:
                sync.dma_start(out=XD[:, coff[i]:coff[i] + cw[i]],
                               in_=xd_in[i][:, :]).then_inc(sch[i], 16)
            sync.dma_start(out=DN[:, :], in_=dn_in[:, :]).then_inc(sn, 16)

        @block.scalar
        def _(scalar):
            for i in scal_chunks:
                scalar.dma_start(out=XD[:, coff[i]:coff[i] + cw[i]],
                                 in_=xd_in[i][:, :]).then_inc(sch[i], 16)

        # DVE: per-node epilogue
        @block.vector
        def _(vector):
            # epilogue after PE fold
            vector.wait_ge(st, 1)
            vector.wait_ge(sn, 16)
            if not layer1:
                # out = dinv * F (+ b2)
                if b2 != 0.0:
                    v_inc(vector.tensor_tensor(
                        out=T[:, :], in0=DN[:, :], in1=F[:, :],
                        op=mybir.AluOpType.mult))
                    v_inc(vector.tensor_scalar_add(W[:, :], T[:, :], float(b2)))
                else:
                    v_inc(vector.tensor_tensor(
                        out=W[:, :], in0=DN[:, :], in1=F[:, :],
                        op=mybir.AluOpType.mult))
            else:
                # s = dinv * F
                v_inc(vector.tensor_tensor(
                    out=S[:, :], in0=DN[:, :], in1=F[:, :],
                    op=mybir.AluOpType.mult))
                if terms is None:
                    # z = (A-B)*relu(s) + B*s;  (A-B)*relu(s) == clamp((A-B)s, 0)
                    if A == B:
                        v_inc(vector.tensor_scalar_mul(T[:, :], S[:, :],
                                                       float(B)))
                    else:
                        clamp = (mybir.AluOpType.max if A - B > 0
                                 else mybir.AluOpType.min)
                        v_inc(vector.tensor_scalar(
                            T[:, :], S[:, :], float(A - B), 0.0,
                            mybir.AluOpType.mult, clamp))
                        if B != 0.0:
                            v_inc(vector.scalar_tensor_tensor(
                                out=T[:, :], in0=S[:, :], scalar=float(B),
                                in1=T[:, :],
                                op0=mybir.AluOpType.mult,
                                op1=mybir.AluOpType.add))
                else:
                    v_inc(vector.memset(T[:, :], 0.0))
                    for (w1k, b1k, w2k) in terms:
                        v_inc(vector.tensor_scalar(
                            U[:, :], S[:, :], float(w1k), float(b1k),
                            mybir.AluOpType.mult, mybir.AluOpType.add))
                        v_inc(vector.tensor_scalar_max(U[:, :], U[:, :], 0.0))
                        v_inc(vector.scalar_tensor_tensor(
                            out=T[:, :], in0=U[:, :], scalar=float(w2k),
                            in1=T[:, :],
                            op0=mybir.AluOpType.mult, op1=mybir.AluOpType.add))
                # w = dinv * z
                v_inc(vector.tensor_tensor(
                    out=W[:, :], in0=DN[:, :], in1=T[:, :],
                    op=mybir.AluOpType.mult))

        # PE: accumulating identity-matmul fold, variable-width rounds,
        # chunks in landing order
        @block.tensor
        def _(tensor):
            tensor.wait_ge(si, 1)
            nr = 0
            inst = None
            for i in order:
                r0, r1 = chunks[i]
                tensor.wait_ge(sch[i], 16)
                for r in range(r0, r1):
                    a = int(offs_r[r])
                    w = int(w_r[r])
                    inst = tensor.matmul(
                        out=F[:, 0:w],
                        lhsT=ID[:, :],
                        rhs=XD[:, a:a + w],
                        start=(nr == 0),
                        stop=(nr == R - 1),
                    )
                    nr += 1
            inst.then_inc(st, 1)

        # final store issued from sync after epilogue completes
        @block.sync
        def _(sync):
            sync.wait_ge(sv, sv_n[0])
            sync.dma_start(out=out_ext[:, :], in_=W[:, :]).then_inc(so, 16)

    return nc


def _pack_chunks(tab16, idx_lay, offs_r, chunks):
    """Build the per-chunk packed DRAM arrays for one core."""
    return {f"xd{i}": np.ascontiguousarray(
        tab16[idx_lay[:, int(offs_r[r0]):int(offs_r[r1])]])
        for i, (r0, r1) in enumerate(chunks)}


def kernel(x, edge_index, W1, b1, W2, b2):
    global LAST_RESULTS
    (idx_c, dinv_own_c, rank_c, dinv_g, R, w_r, offs_r,
     chunks) = _preprocess(x, edge_index)

    w1 = np.asarray(W1, dtype=np.float64).reshape(-1)
    w2 = np.asarray(W2, dtype=np.float64).reshape(-1)
    b1v = np.asarray(b1, dtype=np.float64).reshape(-1)
    b2v = float(np.asarray(b2, dtype=np.float64).reshape(-1)[0])
    if np.all(b1v == 0.0):
        A = float(np.sum(w2 * w1 * (w1 > 0)))
        B = float(np.sum(w2 * w1 * (w1 < 0)))
        terms = None
    else:
        A = B = 0.0
        terms = [(float(w1[k]), float(b1v[k]), float(w2[k]))
                 for k in range(len(w1))]

    # routed tables in permuted (per-core degree-sorted) order + sentinel 0
    x_tab = np.zeros(SENT + 1, dtype=np.float32)
    dinv_tab = np.zeros(SENT + 1, dtype=np.float32)
    xg = np.asarray(x, dtype=np.float32).reshape(-1)
    for c in range(NCORES):
        lo, hi = c * NPC, min((c + 1) * NPC, N)
        nreal = hi - lo
        xv = np.zeros(NPC, dtype=np.float32)
        xv[:nreal] = xg[lo:hi]
        dv = np.zeros(NPC, dtype=np.float32)
        dv[:nreal] = dinv_g[lo:hi]
        order = np.empty(NPC, dtype=np.int64)
        order[rank_c[c]] = np.arange(NPC)
        x_tab[c * NPC:(c + 1) * NPC] = xv[order]
        dinv_tab[c * NPC:(c + 1) * NPC] = dv[order]
    # pre-normalized message table: y_j = dinv_j * x_j (per-node prep on the
    # host, like the degree tables; the per-edge work stays on device)
    y_tab16 = (x_tab * dinv_tab).astype(np.float16)

    trace = bool(os.environ.get("BASS_TRACE"))

    # ---- layer 1 ----
    nc1 = _build(R, w_r, offs_r, chunks, layer1=True, A=A, B=B,
                 terms=terms, out_fp16=True)
    maps1 = []
    for c in range(NCORES):
        m = _pack_chunks(y_tab16, idx_c[c], offs_r, chunks)
        m["dn"] = dinv_own_c[c]
        maps1.append(m)
    res1 = run_bass_kernel_spmd(nc1, maps1, list(range(NCORES)), trace=trace)

    # host routes layer-1 message values to edge slots (halo exchange)
    w_tab16 = np.zeros(SENT + 1, dtype=np.float16)
    for c in range(NCORES):
        w = np.asarray(res1.results[c]["out"])  # [P, CPN], rank j = q*P + p
        w_tab16[c * NPC:(c + 1) * NPC] = w.T.reshape(-1)

    # ---- layer 2 ----
    nc2 = _build(R, w_r, offs_r, chunks, layer1=False, b2=b2v,
                 out_fp16=True)
    maps2 = []
    for c in range(NCORES):
        m = _pack_chunks(w_tab16, idx_c[c], offs_r, chunks)
        m["dn"] = dinv_own_c[c]
        maps2.append(m)
    res2 = run_bass_kernel_spmd(nc2, maps2, list(range(NCORES)), trace=trace)

    LAST_RESULTS = [res1, res2]

    out = np.empty((N, 1), dtype=np.float32)
    for c in range(NCORES):
        lo, hi = c * NPC, min((c + 1) * NPC, N)
        o_sorted = np.asarray(res2.results[c]["out"]).T.reshape(NPC)
        out[lo:hi, 0] = o_sorted[rank_c[c][:hi - lo]]
    return out
